# revision 1
# baseline (speedup 1.0000x reference)
"""Two-layer GAT (single-head, PyG-style) + link predictor on 8 TRN2 NeuronCores.

Strategy (memory-regime):
  - Nodes are sharded 8-way (6250/core, padded to 6272 = 49 windows of 128).
  - Edges are assigned to the core owning their dst node and sorted by dst, so
    edge-softmax and the weighted scatter-sum are core-local.
  - Source features for non-self edges are fetched 128 rows/call with indirect
    DMA row gathers (HW supports one row index per partition per call; the
    ~1.1us/call Q7 descriptor-emission floor is the kernel's bottleneck).
    Self-loop rows are shard-local and streamed sequentially instead.
  - Segment softmax + weighted segment-sum run as one-hot matmuls on the PE:
        psum[d, :] += sum_e p_e * [dst_e == d] * table[src_e, :]
    where the fp16 gather table carries a constant 1.0 tail column so the same
    matmul accumulates the softmax denominator; a per-window epilogue
    normalizes in fp32. exp() needs no segment-max shift (logits are O(6) and
    the shift cancels exactly in the ratio).
  - Dense projections run sharded on PE in fp16; the per-node attention dots
    es = h@a_s / ed = h@a_d come for free as two extra matmul columns
    [W | W@a_s | W@a_d] (the folded vectors are computed on device once).
  - Full-feature fp16 tables for the gathers (halo exchange) are re-assembled
    and replicated between launches on the host (index-space movement only;
    all floating-point math happens on device).

Launches: L1 proj1 -> L2 agg1 -> L3 proj2 -> L4 agg2 -> L5 link predictor.
"""
import time

import numpy as np

import concourse.bass as bass
import concourse.mybir as mybir
import concourse.tile as tile
from concourse import bacc
from concourse.bass_utils import run_bass_kernel_spmd

F32 = mybir.dt.float32
F16 = mybir.dt.float16
I32 = mybir.dt.int32

NCORES = 8
N, F_IN, H, C = 50000, 128, 256, 1
NS = N // NCORES            # 6250 nodes per shard
W = (NS + 127) // 128       # 49 windows per shard
NSP = W * 128               # 6272 padded slots
NEG = -1.0e30               # pad-edge sentinel (exp -> exactly 0)

LAST_EXEC_NS = {}           # launch name -> exec_time_ns (filled per kernel() call)
_PROG_CACHE = {}


# ----------------------------------------------------------------- host prep
def _prep_graph(edge_index):
    """Partition non-self edges by dst shard, sort by dst, window-pad to a
    common per-window tile count across cores. Self-loops are handled by a
    separate sequential stream in the aggregation launch. Edge slot s in the
    [128, T] layout is (t, p) = (s // 128, s % 128)."""
    src = np.asarray(edge_index[0], np.int64)
    dst = np.asarray(edge_index[1], np.int64)

    core = dst // NS
    order = np.argsort(dst, kind="stable")
    src, dst, core = src[order], dst[order], core[order]

    e_src, e_dstloc = [], []
    for c in range(NCORES):
        m = core == c
        e_src.append(src[m])
        e_dstloc.append(dst[m] - c * NS)

    wt = np.zeros(W, dtype=np.int64)
    for c in range(NCORES):
        cnt = np.bincount(e_dstloc[c] // 128, minlength=W)
        wt = np.maximum(wt, (cnt + 127) // 128)
    T = int(wt.sum())

    srcs = np.zeros((NCORES, 128, T), dtype=np.int32)
    dstg = np.zeros((NCORES, 128, T), dtype=np.int32)
    dstf = np.full((NCORES, 128, T), -1.0, dtype=np.float32)
    kind = np.ones((NCORES, 128, T), dtype=np.int8)      # 0 real 1 pad

    wstart = np.concatenate([[0], np.cumsum(wt)]).astype(np.int64)
    for c in range(NCORES):
        win = e_dstloc[c] // 128
        for w in range(W):
            m = win == w
            s = e_src[c][m]
            dl = e_dstloc[c][m]
            n_e = len(s)
            assert n_e <= int(wt[w]) * 128
            t0 = int(wstart[w])
            sl = np.arange(n_e)
            tt, pp = t0 + sl // 128, sl % 128
            srcs[c, pp, tt] = s
            dstg[c, pp, tt] = (dl + c * NS).astype(np.int32)
            dstf[c, pp, tt] = (dl - 128 * w).astype(np.float32)
            kind[c, pp, tt] = 0
    return dict(srcs=srcs, dstg=dstg, dstf=dstf, kind=kind, wt=wt, T=T)


def _expand(es_full, ed_full, g, c):
    """Host halo expansion: per-edge es[src], ed[dst] (+sentinel for pads),
    and per-node self-loop es/ed in [128, W] layout."""
    esx = es_full[g["srcs"][c]].astype(np.float32)
    edx = ed_full[np.minimum(g["dstg"][c], N - 1)].astype(np.float32)
    pad = g["kind"][c] == 1
    esx[pad] = NEG
    edx[pad] = 0.0
    nid = np.arange(NSP)
    nglob = np.minimum(c * NS + nid, N - 1)
    ess = np.where(nid < NS, es_full[nglob], 0.0).astype(np.float32)
    eds = np.where(nid < NS, ed_full[nglob], 0.0).astype(np.float32)
    return esx, edx, ess.reshape(W, 128).T.copy(), eds.reshape(W, 128).T.copy()


# ------------------------------------------------------------- bass programs
def _build_proj(kc, d_out):
    """Projection: psum = x @ [W | W@a_s | W@a_d] per 128-node window.
    Inputs: xT fp16 [kc, W, 128, 128] (pre-tiled transposed features),
            Wm fp16 [kc*128, d_out], asr/adr fp32 [128, d_out].
    Outputs: h16 [NSP, d_out+1] fp16 (features + 1.0 col), es/ed [128, W] f32."""
    nc = bacc.Bacc(num_devices=NCORES)
    xT = nc.dram_tensor("xT", [kc, W, 128, 128], F16, kind="ExternalInput").ap()
    Wm = nc.dram_tensor("Wm", [kc * 128, d_out], F16, kind="ExternalInput").ap()
    asr = nc.dram_tensor("asr", [128, d_out], F32, kind="ExternalInput").ap()
    adr = nc.dram_tensor("adr", [128, d_out], F32, kind="ExternalInput").ap()
    h16 = nc.dram_tensor("h16", [NSP, d_out + 1], F16, kind="ExternalOutput").ap()
    es = nc.dram_tensor("es", [128, W], F32, kind="ExternalOutput").ap()
    ed = nc.dram_tensor("ed", [128, W], F32, kind="ExternalOutput").ap()

    with tile.TileContext(nc) as tc:
        with (
            tc.tile_pool(name="const", bufs=1) as cpool,
            tc.tile_pool(name="x", bufs=6) as xpool,
            tc.tile_pool(name="o", bufs=4) as opool,
            tc.tile_pool(name="ps", bufs=4, space="PSUM") as pspool,
            tc.tile_pool(name="sc", bufs=4) as scpool,
        ):
            asb = cpool.tile([128, d_out], F32)
            nc.sync.dma_start(out=asb[:], in_=asr[:])
            adb = cpool.tile([128, d_out], F32)
            nc.sync.dma_start(out=adb[:], in_=adr[:])
            essb = cpool.tile([128, W], F32)
            edsb = cpool.tile([128, W], F32)

            wsb = []
            for k in range(kc):
                wk = cpool.tile([128, d_out + 2], F16, tag=f"w{k}")
                nc.sync.dma_start(
                    out=wk[:, 0:d_out], in_=Wm[128 * k:128 * (k + 1), :]
                )
                # fold the attention dot vectors in as two extra columns:
                # w_es = W @ a_s (row-wise mul + reduce in f32, cast to f16)
                scr = scpool.tile([128, d_out], F32, tag="wes")
                nc.vector.tensor_tensor(
                    out=scr[:], in0=wk[:, 0:d_out], in1=asb[:],
                    op=mybir.AluOpType.mult,
                )
                wes = scpool.tile([128, 1], F32, tag="wesc")
                nc.vector.reduce_sum(
                    out=wes[:], in_=scr[:], axis=mybir.AxisListType.X
                )
                nc.vector.tensor_copy(out=wk[:, d_out:d_out + 1], in_=wes[:])
                scr2 = scpool.tile([128, d_out], F32, tag="wed")
                nc.vector.tensor_tensor(
                    out=scr2[:], in0=wk[:, 0:d_out], in1=adb[:],
                    op=mybir.AluOpType.mult,
                )
                wed = scpool.tile([128, 1], F32, tag="wedc")
                nc.vector.reduce_sum(
                    out=wed[:], in_=scr2[:], axis=mybir.AxisListType.X
                )
                nc.vector.tensor_copy(out=wk[:, d_out + 1:d_out + 2], in_=wed[:])
                wsb.append(wk)

            for w in range(W):
                ps = pspool.tile([128, d_out + 2], F32, space="PSUM")
                for k in range(kc):
                    xt = xpool.tile([128, 128], F16)
                    nc.sync.dma_start(out=xt[:], in_=xT[k, w])
                    nc.tensor.matmul(
                        out=ps[:], lhsT=xt[:], rhs=wsb[k][:],
                        start=(k == 0), stop=(k == kc - 1),
                    )
                ht = opool.tile([128, d_out + 1], F16)
                nc.vector.tensor_copy(out=ht[:, 0:d_out], in_=ps[:, 0:d_out])
                nc.vector.memset(ht[:, d_out:d_out + 1], 1.0)
                nc.sync.dma_start(out=h16[128 * w:128 * (w + 1), :], in_=ht[:])
                nc.vector.tensor_copy(
                    out=essb[:, w:w + 1], in_=ps[:, d_out:d_out + 1]
                )
                nc.vector.tensor_copy(
                    out=edsb[:, w:w + 1], in_=ps[:, d_out + 1:d_out + 2]
                )
            nc.sync.dma_start(out=es[:], in_=essb[:])
            nc.sync.dma_start(out=ed[:], in_=edsb[:])
    nc.compile()
    return nc


def _build_agg(d, wt, relu):
    """Aggregation launch over one GAT layer (fp16 tables, fp32 softmax).
    Output ho: [NSP, d] fp16 (normalized aggregate + bias (+relu))."""
    T = int(sum(wt))
    nc = bacc.Bacc(num_devices=NCORES)
    table = nc.dram_tensor("table", [N, d + 1], F16, kind="ExternalInput").ap()
    selftab = nc.dram_tensor("selftab", [NSP, d + 1], F16, kind="ExternalInput").ap()
    idx = nc.dram_tensor("idx", [128, T], I32, kind="ExternalInput").ap()
    dstf = nc.dram_tensor("dstf", [128, T], F32, kind="ExternalInput").ap()
    esx = nc.dram_tensor("esx", [128, T], F32, kind="ExternalInput").ap()
    edx = nc.dram_tensor("edx", [128, T], F32, kind="ExternalInput").ap()
    esself = nc.dram_tensor("esself", [128, W], F32, kind="ExternalInput").ap()
    edself = nc.dram_tensor("edself", [128, W], F32, kind="ExternalInput").ap()
    iota = nc.dram_tensor("iota", [128, 128], F32, kind="ExternalInput").ap()
    iotac = nc.dram_tensor("iotac", [128, 1], F32, kind="ExternalInput").ap()
    br = nc.dram_tensor("br", [128, d], F32, kind="ExternalInput").ap()
    ho = nc.dram_tensor("ho", [NSP, d], F16, kind="ExternalOutput").ap()

    with tile.TileContext(nc) as tc:
        with (
            tc.tile_pool(name="const", bufs=1) as cpool,
            tc.tile_pool(name="g", bufs=16) as gpool,
            tc.tile_pool(name="sf", bufs=4) as sfpool,
            tc.tile_pool(name="s", bufs=8) as spool,
            tc.tile_pool(name="o", bufs=3) as opool,
            tc.tile_pool(name="cl", bufs=6) as clpool,
            tc.tile_pool(name="ps", bufs=4, space="PSUM") as pspool,
        ):
            idxs = cpool.tile([128, T], I32)
            nc.sync.dma_start(out=idxs[:], in_=idx[:])
            dsts = cpool.tile([128, T], F32)
            nc.sync.dma_start(out=dsts[:], in_=dstf[:])
            esxs = cpool.tile([128, T], F32)
            nc.sync.dma_start(out=esxs[:], in_=esx[:])
            edxs = cpool.tile([128, T], F32)
            nc.sync.dma_start(out=edxs[:], in_=edx[:])
            esss = cpool.tile([128, W], F32)
            nc.sync.dma_start(out=esss[:], in_=esself[:])
            edss = cpool.tile([128, W], F32)
            nc.sync.dma_start(out=edss[:], in_=edself[:])
            iosb = cpool.tile([128, 128], F32)
            nc.sync.dma_start(out=iosb[:], in_=iota[:])
            iocs = cpool.tile([128, 1], F32)
            nc.sync.dma_start(out=iocs[:], in_=iotac[:])
            brs = cpool.tile([128, d], F32)
            nc.sync.dma_start(out=brs[:], in_=br[:])

            def softmax_weights(es_t, ed_t, cols, tagp):
                lg = cpool.tile([128, cols], F32, tag=f"lg{tagp}")
                nc.vector.tensor_tensor(
                    out=lg[:], in0=es_t[:], in1=ed_t[:], op=mybir.AluOpType.add
                )
                lg2 = cpool.tile([128, cols], F32, tag=f"lg2{tagp}")
                nc.vector.tensor_scalar_mul(out=lg2[:], in0=lg[:], scalar1=0.2)
                nc.vector.tensor_tensor(
                    out=lg[:], in0=lg[:], in1=lg2[:], op=mybir.AluOpType.max
                )
                p = cpool.tile([128, cols], F32, tag=f"p{tagp}")
                nc.scalar.activation(
                    out=p[:], in_=lg[:], func=mybir.ActivationFunctionType.Exp
                )
                return p

            p_all = softmax_weights(esxs, edxs, T, "e")
            p_self = softmax_weights(esss, edss, W, "s")

            t = 0
            for w in range(W):
                ps = pspool.tile([128, d + 1], F32, space="PSUM")
                st = sfpool.tile([128, d + 1], F16)
                nc.sync.dma_start(
                    out=st[:], in_=selftab[128 * w:128 * (w + 1), :]
                )
                sd = spool.tile([128, 128], F16, tag="sdiag")
                nc.vector.scalar_tensor_tensor(
                    out=sd[:], in0=iosb[:], scalar=iocs[:, :1],
                    in1=p_self[:, w:w + 1].to_broadcast([128, 128]),
                    op0=mybir.AluOpType.is_equal, op1=mybir.AluOpType.mult,
                )
                nc.tensor.matmul(
                    out=ps[:], lhsT=sd[:], rhs=st[:],
                    start=True, stop=(int(wt[w]) == 0),
                )
                for i in range(int(wt[w])):
                    gt = gpool.tile([128, d + 1], F16, tag="gather")
                    nc.gpsimd.indirect_dma_start(
                        out=gt[:], out_offset=None, in_=table[:],
                        in_offset=bass.IndirectOffsetOnAxis(
                            ap=idxs[:, t:t + 1], axis=0
                        ),
                    )
                    sp = spool.tile([128, 128], F16, tag="sedge")
                    nc.vector.scalar_tensor_tensor(
                        out=sp[:], in0=iosb[:], scalar=dsts[:, t:t + 1],
                        in1=p_all[:, t:t + 1].to_broadcast([128, 128]),
                        op0=mybir.AluOpType.is_equal, op1=mybir.AluOpType.mult,
                    )
                    nc.tensor.matmul(
                        out=ps[:], lhsT=sp[:], rhs=gt[:],
                        start=False, stop=(i == int(wt[w]) - 1),
                    )
                    t += 1
                rec = clpool.tile([128, 1], F32)
                nc.vector.reciprocal(rec[:], ps[:, d:d + 1])
                ot = opool.tile([128, d], F32)
                nc.vector.tensor_scalar_mul(out=ot[:], in0=ps[:, 0:d], scalar1=rec[:])
                ot16 = opool.tile([128, d], F16, tag="o16")
                if relu:
                    nc.vector.tensor_tensor(
                        out=ot[:], in0=ot[:], in1=brs[:], op=mybir.AluOpType.add
                    )
                    nc.vector.tensor_scalar_max(out=ot16[:], in0=ot[:], scalar1=0.0)
                else:
                    nc.vector.tensor_tensor(
                        out=ot16[:], in0=ot[:], in1=brs[:], op=mybir.AluOpType.add
                    )
                nc.sync.dma_start(out=ho[128 * w:128 * (w + 1), :], in_=ot16[:])
    nc.compile()
    return nc


def _build_link(pt):
    """Link predictor: sigmoid(h2[m0]@wl0 + h2[m1]@wl1 + bl) for pt*128 pairs."""
    nc = bacc.Bacc(num_devices=NCORES)
    table = nc.dram_tensor("table", [N, F_IN], F16, kind="ExternalInput").ap()
    m0 = nc.dram_tensor("m0", [128, pt], I32, kind="ExternalInput").ap()
    m1 = nc.dram_tensor("m1", [128, pt], I32, kind="ExternalInput").ap()
    wl0 = nc.dram_tensor("wl0", [128, F_IN], F32, kind="ExternalInput").ap()
    wl1 = nc.dram_tensor("wl1", [128, F_IN], F32, kind="ExternalInput").ap()
    blr = nc.dram_tensor("blr", [128, 1], F32, kind="ExternalInput").ap()
    z = nc.dram_tensor("z", [128, pt], F32, kind="ExternalOutput").ap()

    with tile.TileContext(nc) as tc:
        with (
            tc.tile_pool(name="const", bufs=1) as cpool,
            tc.tile_pool(name="g", bufs=8) as gpool,
            tc.tile_pool(name="sc", bufs=6) as scpool,
        ):
            m0s = cpool.tile([128, pt], I32)
            nc.sync.dma_start(out=m0s[:], in_=m0[:])
            m1s = cpool.tile([128, pt], I32)
            nc.sync.dma_start(out=m1s[:], in_=m1[:])
            w0s = cpool.tile([128, F_IN], F32)
            nc.sync.dma_start(out=w0s[:], in_=wl0[:])
            w1s = cpool.tile([128, F_IN], F32)
            nc.sync.dma_start(out=w1s[:], in_=wl1[:])
            bls = cpool.tile([128, 1], F32)
            nc.sync.dma_start(out=bls[:], in_=blr[:])
            zsb = cpool.tile([128, pt], F32)

            for t in range(pt):
                g0 = gpool.tile([128, F_IN], F16, tag="g0")
                nc.gpsimd.indirect_dma_start(
                    out=g0[:], out_offset=None, in_=table[:],
                    in_offset=bass.IndirectOffsetOnAxis(ap=m0s[:, t:t + 1], axis=0),
                )
                g1 = gpool.tile([128, F_IN], F16, tag="g1")
                nc.gpsimd.indirect_dma_start(
                    out=g1[:], out_offset=None, in_=table[:],
                    in_offset=bass.IndirectOffsetOnAxis(ap=m1s[:, t:t + 1], axis=0),
                )
                s0 = scpool.tile([128, 1], F32)
                scr = scpool.tile([128, F_IN], F32, tag="scr")
                nc.vector.tensor_tensor(
                    out=scr[:], in0=g0[:], in1=w0s[:], op=mybir.AluOpType.mult
                )
                nc.vector.reduce_sum(out=s0[:], in_=scr[:], axis=mybir.AxisListType.X)
                s1 = scpool.tile([128, 1], F32)
                scr2 = scpool.tile([128, F_IN], F32, tag="scr")
                nc.vector.tensor_tensor(
                    out=scr2[:], in0=g1[:], in1=w1s[:], op=mybir.AluOpType.mult
                )
                nc.vector.reduce_sum(out=s1[:], in_=scr2[:], axis=mybir.AxisListType.X)
                ssum = scpool.tile([128, 1], F32)
                nc.vector.tensor_tensor(
                    out=ssum[:], in0=s0[:], in1=s1[:], op=mybir.AluOpType.add
                )
                nc.scalar.activation(
                    out=zsb[:, t:t + 1], in_=ssum[:],
                    func=mybir.ActivationFunctionType.Sigmoid, bias=bls[:, :1],
                )
            nc.sync.dma_start(out=z[:], in_=zsb[:])
    nc.compile()
    return nc


def _run(name, nc, in_maps, trace=True):
    last = None
    for attempt in range(3):
        try:
            res = run_bass_kernel_spmd(
                nc, in_maps, core_ids=list(range(NCORES)), trace=trace
            )
            LAST_EXEC_NS[name] = res.exec_time_ns
            return res.results
        except Exception as e:  # wedged-device retry (clears on re-attempt)
            last = e
            time.sleep(5)
    raise last


def _rep(v, n=128):
    return np.ascontiguousarray(np.broadcast_to(np.asarray(v, np.float32), (n, len(v))))


def _tile_xT(xfull_shards, kc, d_in):
    """list of [NSP, d_in] fp16 per core -> [NCORES, kc, W, 128, 128] fp16."""
    out = np.zeros((NCORES, kc, W, 128, 128), np.float16)
    for c in range(NCORES):
        xt = xfull_shards[c].T  # [d_in, NSP]
        for k in range(kc):
            blk = xt[128 * k:128 * (k + 1)].reshape(128, W, 128)
            out[c, k] = np.transpose(blk, (1, 0, 2))
    return out


# ------------------------------------------------------------------- kernel
def kernel(features, edge_index, mask, W1, a_src1, a_dst1, b1, W2, a_src2,
           a_dst2, b2, Wl, bl):
    features = np.asarray(features, np.float32)
    edge_index = np.asarray(edge_index, np.int32)
    mask = np.asarray(mask, np.int32)
    W1, W2, Wl = (np.asarray(a, np.float32) for a in (W1, W2, Wl))
    a_src1, a_dst1, b1 = (np.asarray(a, np.float32) for a in (a_src1, a_dst1, b1))
    a_src2, a_dst2, b2 = (np.asarray(a, np.float32) for a in (a_src2, a_dst2, b2))
    bl = np.asarray(bl, np.float32)

    g = _prep_graph(edge_index)
    iota = np.ascontiguousarray(
        np.broadcast_to(np.arange(128, dtype=np.float32), (128, 128))
    )
    iotac = np.arange(128, dtype=np.float32).reshape(128, 1)

    key = (g["T"], tuple(int(x) for x in g["wt"]))
    if key not in _PROG_CACHE:
        _PROG_CACHE[key] = dict(
            p1=_build_proj(1, H),
            a1=_build_agg(H, g["wt"], relu=True),
            p2=_build_proj(2, F_IN),
            a2=_build_agg(F_IN, g["wt"], relu=False),
            lk=_build_link((10000 // NCORES + 127) // 128),
        )
    progs = _PROG_CACHE[key]

    # ---- L1: H1 = X @ W1 (sharded), es1/ed1
    xsh = []
    for c in range(NCORES):
        xs = np.zeros((NSP, F_IN), np.float16)
        xs[:NS] = features[c * NS:(c + 1) * NS]
        xsh.append(xs)
    xT1 = _tile_xT(xsh, 1, F_IN)
    W1h = W1.astype(np.float16)
    r1 = _run("p1", progs["p1"], [
        dict(xT=xT1[c], Wm=W1h, asr=_rep(a_src1), adr=_rep(a_dst1))
        for c in range(NCORES)
    ])
    H1e = np.concatenate([r1[c]["h16"][:NS] for c in range(NCORES)])   # [N, H+1] f16
    es1 = np.concatenate([r1[c]["es"].T.ravel()[:NS] for c in range(NCORES)])
    ed1 = np.concatenate([r1[c]["ed"].T.ravel()[:NS] for c in range(NCORES)])

    # ---- L2: aggregate layer 1 -> h1r = relu(agg + b1)
    b1r = _rep(b1)
    ins2 = []
    for c in range(NCORES):
        esx, edx, ess, eds = _expand(es1, ed1, g, c)
        st = np.zeros((NSP, H + 1), np.float16)
        st[:NS] = H1e[c * NS:(c + 1) * NS]
        ins2.append(dict(table=H1e, selftab=st, idx=g["srcs"][c], dstf=g["dstf"][c],
                         esx=esx, edx=edx, esself=ess, edself=eds,
                         iota=iota, iotac=iotac, br=b1r))
    r2 = _run("a1", progs["a1"], ins2)
    h1r = [r2[c]["ho"] for c in range(NCORES)]                         # [NSP, H] f16

    # ---- L3: H2 = h1r @ W2, es2/ed2
    xT2 = _tile_xT(h1r, 2, H)
    W2h = W2.astype(np.float16)
    r3 = _run("p2", progs["p2"], [
        dict(xT=xT2[c], Wm=W2h, asr=_rep(a_src2), adr=_rep(a_dst2))
        for c in range(NCORES)
    ])
    H2e = np.concatenate([r3[c]["h16"][:NS] for c in range(NCORES)])   # [N, F+1] f16
    es2 = np.concatenate([r3[c]["es"].T.ravel()[:NS] for c in range(NCORES)])
    ed2 = np.concatenate([r3[c]["ed"].T.ravel()[:NS] for c in range(NCORES)])

    # ---- L4: aggregate layer 2 -> h2 = agg + b2
    b2r = _rep(b2)
    ins4 = []
    for c in range(NCORES):
        esx, edx, ess, eds = _expand(es2, ed2, g, c)
        st = np.zeros((NSP, F_IN + 1), np.float16)
        st[:NS] = H2e[c * NS:(c + 1) * NS]
        ins4.append(dict(table=H2e, selftab=st, idx=g["srcs"][c], dstf=g["dstf"][c],
                         esx=esx, edx=edx, esself=ess, edself=eds,
                         iota=iota, iotac=iotac, br=b2r))
    r4 = _run("a2", progs["a2"], ins4)
    h2 = np.concatenate([r4[c]["ho"][:NS] for c in range(NCORES)])     # [N, F] f16

    # ---- L5: link predictor
    P = mask.shape[0]
    pc = P // NCORES
    pt = (pc + 127) // 128
    m0 = np.zeros((NCORES, 128, pt), np.int32)
    m1 = np.zeros((NCORES, 128, pt), np.int32)
    mT = mask.T
    for c in range(NCORES):
        s = np.arange(pc)
        m0[c, s % 128, s // 128] = mT[0][c * pc:(c + 1) * pc]
        m1[c, s % 128, s // 128] = mT[1][c * pc:(c + 1) * pc]
    wl0 = _rep(Wl[:F_IN, 0])
    wl1 = _rep(Wl[F_IN:, 0])
    blr = np.full((128, 1), float(bl[0]), np.float32)
    r5 = _run("lk", progs["lk"], [
        dict(table=h2, m0=m0[c], m1=m1[c], wl0=wl0, wl1=wl1, blr=blr)
        for c in range(NCORES)
    ])
    out = np.zeros((P, 1), np.float32)
    for c in range(NCORES):
        s = np.arange(pc)
        out[c * pc:(c + 1) * pc, 0] = r5[c]["z"][s % 128, s // 128]

    tot = sum(v for v in LAST_EXEC_NS.values() if v)
    print(f"kernel launches ns: {LAST_EXEC_NS} total {tot}")
    return out



# revision 5
# speedup vs baseline: 2.1074x; 2.1074x over previous
"""Two-layer GAT (single-head, PyG-style) + link predictor on 8 TRN2 NeuronCores.

Strategy (memory-regime):
  - Nodes sharded 8-way (6250/core, padded to 6272 = 49 windows of 128); edges
    assigned to the core owning their dst node so edge-softmax and the weighted
    scatter-sum are core-local.
  - Source features for non-self edges are fetched with batched dma_gather
    (InstDMAGatherAnt): 1024 rows per call, calls round-robined over 4 SWDGE
    queues. Gather indices are int16, so the halo table is split into two
    row-halves of 25024 rows; edge slots are partitioned by source half.
  - Segment softmax + weighted segment-sum run as one-hot matmuls on the PE:
        psum[dloc, :] += sum_slot p_slot * [dst_slot == dloc] * gt[slot, :]
    with a second 1-column matmul against a ones vector accumulating the
    softmax denominator. exp() needs no segment-max shift (logits are O(6) and
    the shift cancels in the ratio). Self-loop feature rows are shard-local
    and streamed contiguously; their diag(p_self) matmul opens each window.
  - Dense projections run sharded on PE in fp16; the per-node attention dots
    es = h@a_s / ed = h@a_d come as two extra matmul columns [W | W@a_s | W@a_d]
    (folded on device once per launch).
  - All host work between launches is index-space movement / layout shuffling
    (fp16 byte moves, int index prep); per-edge and per-node float math (exp,
    leaky-relu, softmax, dots) happens on device.

Launches: L1 proj1 -> L2 agg1 -> L3 proj2 -> L4 agg2 -> L5 link predictor.
"""
import time

import numpy as np

import concourse.bass as bass
import concourse.mybir as mybir
import concourse.tile as tile
from concourse import bacc
from concourse.bass_utils import run_bass_kernel_spmd

F32 = mybir.dt.float32
F16 = mybir.dt.float16
I16 = mybir.dt.int16
I32 = mybir.dt.int32

NCORES = 8
N, F_IN, H, C = 50000, 128, 256, 1
NS = N // NCORES            # 6250 nodes per shard
W = (NS + 127) // 128       # 49 windows per shard
NSP = W * 128               # 6272 padded slots
RH = 25024                  # rows per half table (int16-indexable)
NPAD = 2 * RH               # 50048 padded global rows
NEG = -1.0e30               # pad-edge sentinel (exp -> exactly 0)
CH = 8                      # tiles per dma_gather call (1024 rows = ring max)
NQ = 4                      # SWDGE queues
LOOKAHEAD = 2               # windows of gather prefetch
GBUFS = 6                   # gather ring buffers per region

LAST_EXEC_NS = {}           # launch name -> exec_time_ns (filled per kernel() call)
_PROG_CACHE = {}


# ----------------------------------------------------------------- host prep
def _prep_graph(edge_index):
    """Partition non-self edges by dst shard, split by src half, sort by dst.
    Slots are tile-aligned per (window, half): window w's half-R edges occupy
    tiles [off_R(w), off_R(w)+nt_R[w]) of region R's tile space; slot (t, p)
    holds one edge.  Counts are maxed across cores for a shared SPMD shape."""
    src = np.asarray(edge_index[0], np.int64)
    dst = np.asarray(edge_index[1], np.int64)
    core = dst // NS

    ntA = np.zeros(W, np.int64)
    ntB = np.zeros(W, np.int64)
    per_core = []
    for c in range(NCORES):
        m = core == c
        s, dl = src[m], dst[m] - c * NS
        half = (s >= RH).astype(np.int64)
        w = dl // 128
        order = np.lexsort((dl, half * W + w))
        s, dl, half, w = s[order], dl[order], half[order], w[order]
        per_core.append((s, dl, half, w))
        for r, nt in ((0, ntA), (1, ntB)):
            cnt = np.bincount(w[half == r], minlength=W)
            nt[:] = np.maximum(nt, (cnt + 127) // 128)
    TA, TB = int(ntA.sum()), int(ntB.sum())
    NT = TA + TB
    offA = np.concatenate([[0], np.cumsum(ntA)]).astype(np.int64)
    offB = np.concatenate([[0], np.cumsum(ntB)]).astype(np.int64)

    idxA = np.zeros((NCORES, 16, TA * 8), np.int16)
    idxB = np.zeros((NCORES, 16, TB * 8), np.int16)
    dstf = np.zeros((NCORES, 128, NT), np.float32)
    esrc = np.zeros((NCORES, 128, NT), np.int32)   # src node per slot (or -1 pad)
    edst = np.zeros((NCORES, 128, NT), np.int32)   # dst node (global) per slot
    esrc[:] = -1
    for c in range(NCORES):
        s, dl, half, w = per_core[c]
        for r, idx, toff, base, nt in ((0, idxA, offA, 0, ntA),
                                       (1, idxB, offB, TA, ntB)):
            m = half == r
            sr, dlr, wr = s[m], dl[m], w[m]
            for wi in range(W):
                mw = wr == wi
                sw, dw = sr[mw], dlr[mw]
                n_e = len(sw)
                slot = toff[wi] * 128 + np.arange(n_e)
                tt, pp = slot // 128, slot % 128
                idx[c, slot % 16, slot // 16] = (sw - r * RH).astype(np.int16)
                gcol = base + tt
                dstf[c, pp, gcol] = (dw - 128 * wi).astype(np.float32)
                esrc[c, pp, gcol] = sw
                edst[c, pp, gcol] = dw + c * NS
    idxA = np.broadcast_to(idxA[:, None], (NCORES, 8, 16, TA * 8)).reshape(
        NCORES, 128, TA * 8)
    idxB = np.broadcast_to(idxB[:, None], (NCORES, 8, 16, TB * 8)).reshape(
        NCORES, 128, TB * 8)
    return dict(ntA=tuple(int(x) for x in ntA), ntB=tuple(int(x) for x in ntB),
                TA=TA, TB=TB, NT=NT,
                offA=offA, offB=offB,
                idxA=np.ascontiguousarray(idxA),
                idxB=np.ascontiguousarray(idxB),
                dstf=dstf, esrc=esrc, edst=edst)


def _expand(es_full, ed_full, g, c):
    """Per-slot es[src], ed[dst] (+NEG sentinel for pads) and per-node
    self-loop es/ed in [128, W] layout.  Pure index-space gathers."""
    pad = g["esrc"][c] < 0
    esx = es_full[np.where(pad, 0, g["esrc"][c])].astype(np.float32)
    edx = ed_full[np.minimum(g["edst"][c], N - 1)].astype(np.float32)
    esx[pad] = NEG
    edx[pad] = 0.0
    nid = np.arange(NSP)
    nglob = np.minimum(c * NS + nid, N - 1)
    ess = np.where(nid < NS, es_full[nglob], 0.0).astype(np.float32)
    eds = np.where(nid < NS, ed_full[nglob], 0.0).astype(np.float32)
    return esx, edx, ess.reshape(W, 128).T.copy(), eds.reshape(W, 128).T.copy()


# ------------------------------------------------------------- bass programs
def _build_proj(kc, d_out):
    """Projection: psum = x @ [W | W@a_s | W@a_d] per 128-node window.
    Inputs: xTf fp16 [128, kc*W*128] (features on partitions; free dim ordered
    [k, w, node]), Wm fp16 [kc*128, d_out], asr/adr fp32 [128, d_out].
    Outputs: hT [128, W*d_out] fp16 (SBUF-native layout), es/ed [128, W]."""
    nc = bacc.Bacc(num_devices=NCORES)
    xTf = nc.dram_tensor("xTf", [128, kc * W * 128], F16, kind="ExternalInput").ap()
    Wm = nc.dram_tensor("Wm", [kc * 128, d_out], F16, kind="ExternalInput").ap()
    asr = nc.dram_tensor("asr", [128, d_out], F32, kind="ExternalInput").ap()
    adr = nc.dram_tensor("adr", [128, d_out], F32, kind="ExternalInput").ap()
    hT = nc.dram_tensor("hT", [128, W * d_out], F16, kind="ExternalOutput").ap()
    es = nc.dram_tensor("es", [128, W], F32, kind="ExternalOutput").ap()
    ed = nc.dram_tensor("ed", [128, W], F32, kind="ExternalOutput").ap()

    with tile.TileContext(nc) as tc:
        with (
            tc.tile_pool(name="const", bufs=1) as cpool,
            tc.tile_pool(name="ps", bufs=4, space="PSUM") as pspool,
            tc.tile_pool(name="sc", bufs=4) as scpool,
        ):
            asb = cpool.tile([128, d_out], F32)
            nc.sync.dma_start(out=asb[:], in_=asr[:])
            adb = cpool.tile([128, d_out], F32)
            nc.sync.dma_start(out=adb[:], in_=adr[:])
            xsb = cpool.tile([128, kc * W * 128], F16)
            for k in range(kc):
                nc.sync.dma_start(
                    out=xsb[:, k * W * 128:(k + 1) * W * 128],
                    in_=xTf[:, k * W * 128:(k + 1) * W * 128],
                )
            essb = cpool.tile([128, W], F32)
            edsb = cpool.tile([128, W], F32)
            obig = cpool.tile([128, W * d_out], F16)

            wsb = []
            for k in range(kc):
                wk = cpool.tile([128, d_out + 2], F16, tag=f"w{k}")
                nc.sync.dma_start(
                    out=wk[:, 0:d_out], in_=Wm[128 * k:128 * (k + 1), :]
                )
                scr = scpool.tile([128, d_out], F32, tag="wes")
                nc.vector.tensor_tensor(
                    out=scr[:], in0=wk[:, 0:d_out], in1=asb[:],
                    op=mybir.AluOpType.mult,
                )
                wes = scpool.tile([128, 1], F32, tag="wesc")
                nc.vector.reduce_sum(out=wes[:], in_=scr[:], axis=mybir.AxisListType.X)
                nc.vector.tensor_copy(out=wk[:, d_out:d_out + 1], in_=wes[:])
                scr2 = scpool.tile([128, d_out], F32, tag="wed")
                nc.vector.tensor_tensor(
                    out=scr2[:], in0=wk[:, 0:d_out], in1=adb[:],
                    op=mybir.AluOpType.mult,
                )
                wed = scpool.tile([128, 1], F32, tag="wedc")
                nc.vector.reduce_sum(out=wed[:], in_=scr2[:], axis=mybir.AxisListType.X)
                nc.vector.tensor_copy(out=wk[:, d_out + 1:d_out + 2], in_=wed[:])
                wsb.append(wk)

            for w in range(W):
                ps = pspool.tile([128, d_out + 2], F32, space="PSUM")
                for k in range(kc):
                    nc.tensor.matmul(
                        out=ps[:],
                        lhsT=xsb[:, (k * W + w) * 128:(k * W + w + 1) * 128],
                        rhs=wsb[k][:],
                        start=(k == 0), stop=(k == kc - 1),
                    )
                nc.vector.tensor_copy(
                    out=obig[:, w * d_out:(w + 1) * d_out], in_=ps[:, 0:d_out]
                )
                nc.vector.tensor_copy(out=essb[:, w:w + 1], in_=ps[:, d_out:d_out + 1])
                nc.vector.tensor_copy(
                    out=edsb[:, w:w + 1], in_=ps[:, d_out + 1:d_out + 2]
                )
            nc.sync.dma_start(out=hT[:], in_=obig[:])
            nc.sync.dma_start(out=es[:], in_=essb[:])
            nc.sync.dma_start(out=ed[:], in_=edsb[:])
    nc.compile()
    return nc


def _build_agg(d, ntA, ntB, relu):
    """Aggregation over one GAT layer: batched dma_gather of source rows from
    the two half tables, one-hot matmul scatter per window, fp32 softmax
    normalize + bias (+relu) epilogue.  Output hoT [128, W*d] fp16."""
    TA, TB = sum(ntA), sum(ntB)
    NT = TA + TB
    offA = [0]
    offB = [0]
    for w in range(W):
        offA.append(offA[-1] + ntA[w])
        offB.append(offB[-1] + ntB[w])
    nAc = (TA + CH - 1) // CH
    nBc = (TB + CH - 1) // CH

    nc = bacc.Bacc(num_devices=NCORES, num_swdge_queues=NQ)
    tableA = nc.dram_tensor("tableA", [RH, d], F16, kind="ExternalInput").ap()
    tableB = nc.dram_tensor("tableB", [RH, d], F16, kind="ExternalInput").ap()
    selfT = nc.dram_tensor("selfT", [128, W * d], F16, kind="ExternalInput").ap()
    idxA = nc.dram_tensor("idxA", [128, TA * 8], I16, kind="ExternalInput").ap()
    idxB = nc.dram_tensor("idxB", [128, TB * 8], I16, kind="ExternalInput").ap()
    dstf = nc.dram_tensor("dstf", [128, NT], F32, kind="ExternalInput").ap()
    esx = nc.dram_tensor("esx", [128, NT], F32, kind="ExternalInput").ap()
    edx = nc.dram_tensor("edx", [128, NT], F32, kind="ExternalInput").ap()
    esself = nc.dram_tensor("esself", [128, W], F32, kind="ExternalInput").ap()
    edself = nc.dram_tensor("edself", [128, W], F32, kind="ExternalInput").ap()
    iota = nc.dram_tensor("iota", [128, 128], F32, kind="ExternalInput").ap()
    iotac = nc.dram_tensor("iotac", [128, 1], F32, kind="ExternalInput").ap()
    br = nc.dram_tensor("br", [128, d], F32, kind="ExternalInput").ap()
    ho = nc.dram_tensor("ho", [128, W * d], F16, kind="ExternalOutput").ap()

    with tile.TileContext(nc) as tc:
        with (
            tc.tile_pool(name="const", bufs=1) as cpool,
            tc.tile_pool(name="ga", bufs=GBUFS) as gapool,
            tc.tile_pool(name="gb", bufs=GBUFS) as gbpool,
            tc.tile_pool(name="sp", bufs=8) as sppool,
            tc.tile_pool(name="ep", bufs=4) as eppool,
            tc.tile_pool(name="ps", bufs=4, space="PSUM") as pspool,
            tc.tile_pool(name="psd", bufs=4, space="PSUM") as psdpool,
        ):
            idxAs = cpool.tile([128, TA * 8], I16)
            nc.sync.dma_start(out=idxAs[:], in_=idxA[:])
            idxBs = cpool.tile([128, TB * 8], I16)
            nc.sync.dma_start(out=idxBs[:], in_=idxB[:])
            dsts = cpool.tile([128, NT], F32)
            nc.sync.dma_start(out=dsts[:], in_=dstf[:])
            esxs = cpool.tile([128, NT], F32)
            nc.sync.dma_start(out=esxs[:], in_=esx[:])
            edxs = cpool.tile([128, NT], F32)
            nc.sync.dma_start(out=edxs[:], in_=edx[:])
            esss = cpool.tile([128, W], F32)
            nc.sync.dma_start(out=esss[:], in_=esself[:])
            edss = cpool.tile([128, W], F32)
            nc.sync.dma_start(out=edss[:], in_=edself[:])
            iosb = cpool.tile([128, 128], F32)
            nc.sync.dma_start(out=iosb[:], in_=iota[:])
            iocs = cpool.tile([128, 1], F32)
            nc.sync.dma_start(out=iocs[:], in_=iotac[:])
            brs = cpool.tile([128, d], F32)
            nc.sync.dma_start(out=brs[:], in_=br[:])
            selfs = cpool.tile([128, W * d], F16)
            nc.sync.dma_start(out=selfs[:], in_=selfT[:])
            ones = cpool.tile([128, 1], F16)
            nc.vector.memset(ones[:], 1.0)
            obig = cpool.tile([128, W * d], F16)

            def softmax_weights(es_t, ed_t, cols, tagp):
                lg = cpool.tile([128, cols], F32, tag=f"lg{tagp}")
                nc.vector.tensor_tensor(
                    out=lg[:], in0=es_t[:], in1=ed_t[:], op=mybir.AluOpType.add
                )
                lg2 = cpool.tile([128, cols], F32, tag=f"lg2{tagp}")
                nc.vector.tensor_scalar_mul(out=lg2[:], in0=lg[:], scalar1=0.2)
                nc.vector.tensor_tensor(
                    out=lg[:], in0=lg[:], in1=lg2[:], op=mybir.AluOpType.max
                )
                p = cpool.tile([128, cols], F32, tag=f"p{tagp}")
                nc.scalar.activation(
                    out=p[:], in_=lg[:], func=mybir.ActivationFunctionType.Exp
                )
                return p

            p_all = softmax_weights(esxs, edxs, NT, "e")
            p_self = softmax_weights(esss, edss, W, "s")

            gbufA, gbufB = [], []
            emit = [0, 0]
            qctr = [0]

            def emit_chunk(region):
                k = emit[region]
                tot, pool, idxs, tab, buf = (
                    (TA, gapool, idxAs, tableA, gbufA) if region == 0
                    else (TB, gbpool, idxBs, tableB, gbufB)
                )
                t0 = k * CH
                ntiles = min(CH, tot - t0)
                gt = pool.tile([128, CH, d], F16, tag=f"g{region}")
                nc.gpsimd.dma_gather(
                    out_ap=gt[:, 0:ntiles, :], in_ap=tab[:],
                    idxs_ap=idxs[:, t0 * 8:(t0 + ntiles) * 8],
                    num_idxs=ntiles * 128, num_idxs_reg=ntiles * 128,
                    elem_size=d, queue_num=qctr[0] % NQ,
                )
                qctr[0] += 1
                buf.append(gt)
                emit[region] = k + 1

            def ensure(wtarget):
                needA = (offA[wtarget + 1] + CH - 1) // CH
                needB = (offB[wtarget + 1] + CH - 1) // CH
                while emit[0] < min(needA, nAc) or emit[1] < min(needB, nBc):
                    if emit[0] < min(needA, nAc):
                        emit_chunk(0)
                    if emit[1] < min(needB, nBc):
                        emit_chunk(1)

            for w in range(W):
                ensure(min(w + LOOKAHEAD, W - 1))
                ps = pspool.tile([128, d], F32, space="PSUM")
                psd = psdpool.tile([128, 1], F32, space="PSUM")
                sd = sppool.tile([128, 128], F16, tag="sd")
                nc.vector.scalar_tensor_tensor(
                    out=sd[:], in0=iosb[:], scalar=iocs[:, :1],
                    in1=p_self[:, w:w + 1].to_broadcast([128, 128]),
                    op0=mybir.AluOpType.is_equal, op1=mybir.AluOpType.mult,
                )
                refs = (
                    [(gbufA, t) for t in range(offA[w], offA[w + 1])]
                    + [(gbufB, TA + t) for t in range(offB[w], offB[w + 1])]
                )
                nc.tensor.matmul(
                    out=ps[:, 0:d], lhsT=sd[:],
                    rhs=selfs[:, w * d:(w + 1) * d],
                    start=True, stop=(len(refs) == 0),
                )
                for j, (buf, gcol) in enumerate(refs):
                    t = gcol if gcol < TA else gcol - TA
                    sp = sppool.tile([128, 128], F16, tag="sp")
                    nc.vector.scalar_tensor_tensor(
                        out=sp[:], in0=iosb[:], scalar=dsts[:, gcol:gcol + 1],
                        in1=p_all[:, gcol:gcol + 1].to_broadcast([128, 128]),
                        op0=mybir.AluOpType.is_equal, op1=mybir.AluOpType.mult,
                    )
                    last = j == len(refs) - 1
                    nc.tensor.matmul(
                        out=ps[:, 0:d], lhsT=sp[:],
                        rhs=buf[t // CH][:, t % CH, :],
                        start=False, stop=last,
                    )
                    nc.tensor.matmul(
                        out=psd[:, 0:1], lhsT=sp[:], rhs=ones[:],
                        start=(j == 0), stop=last,
                    )
                den = eppool.tile([128, 1], F32, tag="den")
                if refs:
                    nc.vector.tensor_tensor(
                        out=den[:], in0=psd[:, 0:1], in1=p_self[:, w:w + 1],
                        op=mybir.AluOpType.add,
                    )
                else:
                    nc.vector.tensor_copy(out=den[:], in_=p_self[:, w:w + 1])
                rec = eppool.tile([128, 1], F32, tag="rec")
                nc.vector.reciprocal(rec[:], den[:])
                ot = eppool.tile([128, d], F32, tag="ot")
                nc.vector.tensor_scalar_mul(out=ot[:], in0=ps[:, 0:d], scalar1=rec[:])
                if relu:
                    nc.vector.tensor_tensor(
                        out=ot[:], in0=ot[:], in1=brs[:], op=mybir.AluOpType.add
                    )
                    nc.vector.tensor_scalar_max(
                        out=obig[:, w * d:(w + 1) * d], in0=ot[:], scalar1=0.0
                    )
                else:
                    nc.vector.tensor_tensor(
                        out=obig[:, w * d:(w + 1) * d], in0=ot[:], in1=brs[:],
                        op=mybir.AluOpType.add,
                    )
            nc.sync.dma_start(out=ho[:], in_=obig[:])
    nc.compile()
    return nc


def _build_link(gt_sizes):
    """Link predictor: sigmoid(h2[m0]@wl0 + h2[m1]@wl1 + bl).
    Pairs are grouped [AA, AB, BB, BA] by (m0 half, m1 half); gt_sizes gives
    tiles per group.  m0 gathers: A(AA+AB), B(BB+BA); m1: A(AA), B(AB+BB),
    A(BA)."""
    PT = sum(gt_sizes)
    g0, g1, g2, g3 = gt_sizes
    nc = bacc.Bacc(num_devices=NCORES, num_swdge_queues=NQ)
    tableA = nc.dram_tensor("tableA", [RH, F_IN], F16, kind="ExternalInput").ap()
    tableB = nc.dram_tensor("tableB", [RH, F_IN], F16, kind="ExternalInput").ap()
    i0 = nc.dram_tensor("i0", [128, PT * 8], I16, kind="ExternalInput").ap()
    i1 = nc.dram_tensor("i1", [128, PT * 8], I16, kind="ExternalInput").ap()
    wl0 = nc.dram_tensor("wl0", [128, F_IN], F32, kind="ExternalInput").ap()
    wl1 = nc.dram_tensor("wl1", [128, F_IN], F32, kind="ExternalInput").ap()
    blr = nc.dram_tensor("blr", [128, 1], F32, kind="ExternalInput").ap()
    z = nc.dram_tensor("z", [128, PT], F32, kind="ExternalOutput").ap()

    with tile.TileContext(nc) as tc:
        with (
            tc.tile_pool(name="const", bufs=1) as cpool,
            tc.tile_pool(name="sc", bufs=6) as scpool,
        ):
            i0s = cpool.tile([128, PT * 8], I16)
            nc.sync.dma_start(out=i0s[:], in_=i0[:])
            i1s = cpool.tile([128, PT * 8], I16)
            nc.sync.dma_start(out=i1s[:], in_=i1[:])
            w0s = cpool.tile([128, F_IN], F32)
            nc.sync.dma_start(out=w0s[:], in_=wl0[:])
            w1s = cpool.tile([128, F_IN], F32)
            nc.sync.dma_start(out=w1s[:], in_=wl1[:])
            bls = cpool.tile([128, 1], F32)
            nc.sync.dma_start(out=bls[:], in_=blr[:])
            zsb = cpool.tile([128, PT], F32)
            gb0 = cpool.tile([128, PT, F_IN], F16)
            gb1 = cpool.tile([128, PT, F_IN], F16)

            calls0 = [(0, g0 + g1, tableA), (g0 + g1, PT, tableB)]
            calls1 = [(0, g0, tableA), (g0, g0 + g1 + g2, tableB), (g0 + g1 + g2, PT, tableA)]
            qi = 0
            for (buf, idxs, calls) in ((gb0, i0s, calls0), (gb1, i1s, calls1)):
                for (t0, t1, tab) in calls:
                    if t1 <= t0:
                        continue
                    nc.gpsimd.dma_gather(
                        out_ap=buf[:, t0:t1, :], in_ap=tab[:],
                        idxs_ap=idxs[:, t0 * 8:t1 * 8],
                        num_idxs=(t1 - t0) * 128, num_idxs_reg=(t1 - t0) * 128,
                        elem_size=F_IN, queue_num=qi % NQ,
                    )
                    qi += 1

            for t in range(PT):
                s0 = scpool.tile([128, 1], F32)
                scr = scpool.tile([128, F_IN], F32, tag="scr")
                nc.vector.tensor_tensor(
                    out=scr[:], in0=gb0[:, t, :], in1=w0s[:], op=mybir.AluOpType.mult
                )
                nc.vector.reduce_sum(out=s0[:], in_=scr[:], axis=mybir.AxisListType.X)
                s1 = scpool.tile([128, 1], F32)
                scr2 = scpool.tile([128, F_IN], F32, tag="scr")
                nc.vector.tensor_tensor(
                    out=scr2[:], in0=gb1[:, t, :], in1=w1s[:], op=mybir.AluOpType.mult
                )
                nc.vector.reduce_sum(out=s1[:], in_=scr2[:], axis=mybir.AxisListType.X)
                ssum = scpool.tile([128, 1], F32)
                nc.vector.tensor_tensor(
                    out=ssum[:], in0=s0[:], in1=s1[:], op=mybir.AluOpType.add
                )
                nc.scalar.activation(
                    out=zsb[:, t:t + 1], in_=ssum[:],
                    func=mybir.ActivationFunctionType.Sigmoid, bias=bls[:, :1],
                )
            nc.sync.dma_start(out=z[:], in_=zsb[:])
    nc.compile()
    return nc


def _run(name, nc, in_maps, trace=True):
    last = None
    for attempt in range(3):
        try:
            res = run_bass_kernel_spmd(
                nc, in_maps, core_ids=list(range(NCORES)), trace=trace
            )
            LAST_EXEC_NS[name] = res.exec_time_ns
            return res.results
        except Exception as e:  # wedged-device retry (clears on re-attempt)
            last = e
            time.sleep(5)
    raise last


def _rep(v, n=128):
    return np.ascontiguousarray(np.broadcast_to(np.asarray(v, np.float32), (n, len(v))))


def _shard_xT(xfull):
    """[NPAD-ish node-major, d_in] -> per-core xTf [128, kc*W*128] f16."""
    d_in = xfull.shape[1]
    kc = d_in // 128
    out = np.zeros((NCORES, 128, kc * W * 128), np.float16)
    for c in range(NCORES):
        xs = np.zeros((NSP, d_in), np.float16)
        xs[:NS] = xfull[c * NS:(c + 1) * NS]
        xt = xs.T.reshape(kc, 128, W, 128)       # [k, f, w, node]
        out[c] = xt.reshape(128, -1) if kc == 1 else np.ascontiguousarray(
            xt.transpose(1, 0, 2, 3)).reshape(128, -1)
    return out


def _assemble_full(hT_list, d):
    """Per-core hT [128, W*d] (SBUF layout) -> node-major [NPAD, d] f16."""
    full = np.zeros((NPAD, d), np.float16)
    for c in range(NCORES):
        resh = hT_list[c].reshape(128, W, d).transpose(1, 0, 2).reshape(NSP, d)
        full[c * NS:(c + 1) * NS] = resh[:NS]
    return full


def _es_full(es_list):
    """Per-core es [128, W] -> node-major [N] f32."""
    out = np.zeros(N, np.float32)
    for c in range(NCORES):
        out[c * NS:(c + 1) * NS] = es_list[c].T.ravel()[:NS]
    return out


# ------------------------------------------------------------------- kernel
def kernel(features, edge_index, mask, W1, a_src1, a_dst1, b1, W2, a_src2,
           a_dst2, b2, Wl, bl):
    features = np.asarray(features, np.float32)
    edge_index = np.asarray(edge_index, np.int32)
    mask = np.asarray(mask, np.int32)
    W1, W2, Wl = (np.asarray(a, np.float32) for a in (W1, W2, Wl))
    a_src1, a_dst1, b1 = (np.asarray(a, np.float32) for a in (a_src1, a_dst1, b1))
    a_src2, a_dst2, b2 = (np.asarray(a, np.float32) for a in (a_src2, a_dst2, b2))
    bl = np.asarray(bl, np.float32)

    g = _prep_graph(edge_index)
    iota = np.ascontiguousarray(
        np.broadcast_to(np.arange(128, dtype=np.float32), (128, 128))
    )
    iotac = np.arange(128, dtype=np.float32).reshape(128, 1)

    # ---- link-predictor pair grouping (by m0/m1 half)
    P = mask.shape[0]
    pc = P // NCORES
    mT = mask.T
    lk_group = []
    gcounts = np.zeros((NCORES, 4), np.int64)
    gmap = {(0, 0): 0, (0, 1): 1, (1, 1): 2, (1, 0): 3}
    for c in range(NCORES):
        m0 = mT[0][c * pc:(c + 1) * pc].astype(np.int64)
        m1 = mT[1][c * pc:(c + 1) * pc].astype(np.int64)
        gk = np.array([gmap[(int(a >= RH), int(b >= RH))] for a, b in zip(m0, m1)])
        lk_group.append((m0, m1, gk))
        for q in range(4):
            gcounts[c, q] = int((gk == q).sum())
    gt_sizes = tuple(int((gcounts[:, q].max() + 127) // 128) for q in range(4))
    PT = sum(gt_sizes)

    key = (g["ntA"], g["ntB"], gt_sizes)
    if key not in _PROG_CACHE:
        _PROG_CACHE[key] = dict(
            p1=_build_proj(1, H),
            a1=_build_agg(H, g["ntA"], g["ntB"], relu=True),
            p2=_build_proj(2, F_IN),
            a2=_build_agg(F_IN, g["ntA"], g["ntB"], relu=False),
            lk=_build_link(gt_sizes),
        )
    progs = _PROG_CACHE[key]

    # ---- L1: H1 = X @ W1 (sharded), es1/ed1
    xT1 = _shard_xT(features)
    W1h = W1.astype(np.float16)
    r1 = _run("p1", progs["p1"], [
        dict(xTf=xT1[c], Wm=W1h, asr=_rep(a_src1), adr=_rep(a_dst1))
        for c in range(NCORES)
    ])
    h1T = [r1[c]["hT"] for c in range(NCORES)]
    es1 = _es_full([r1[c]["es"] for c in range(NCORES)])
    ed1 = _es_full([r1[c]["ed"] for c in range(NCORES)])
    h1full = _assemble_full(h1T, H)

    # ---- L2: aggregate layer 1 -> h1r = relu(agg + b1)
    b1r = _rep(b1)
    ins2 = []
    for c in range(NCORES):
        esx, edx, ess, eds = _expand(es1, ed1, g, c)
        ins2.append(dict(tableA=h1full[:RH], tableB=h1full[RH:],
                         selfT=h1T[c], idxA=g["idxA"][c], idxB=g["idxB"][c],
                         dstf=g["dstf"][c], esx=esx, edx=edx,
                         esself=ess, edself=eds, iota=iota, iotac=iotac, br=b1r))
    r2 = _run("a1", progs["a1"], ins2)
    h1rT = [r2[c]["ho"] for c in range(NCORES)]

    # ---- L3: H2 = h1r @ W2, es2/ed2
    h1rfull = _assemble_full(h1rT, H)
    xT2 = _shard_xT(h1rfull)
    W2h = W2.astype(np.float16)
    r3 = _run("p2", progs["p2"], [
        dict(xTf=xT2[c], Wm=W2h, asr=_rep(a_src2), adr=_rep(a_dst2))
        for c in range(NCORES)
    ])
    h2T = [r3[c]["hT"] for c in range(NCORES)]
    es2 = _es_full([r3[c]["es"] for c in range(NCORES)])
    ed2 = _es_full([r3[c]["ed"] for c in range(NCORES)])
    h2full = _assemble_full(h2T, F_IN)

    # ---- L4: aggregate layer 2 -> h2 = agg + b2
    b2r = _rep(b2)
    ins4 = []
    for c in range(NCORES):
        esx, edx, ess, eds = _expand(es2, ed2, g, c)
        ins4.append(dict(tableA=h2full[:RH], tableB=h2full[RH:],
                         selfT=h2T[c], idxA=g["idxA"][c], idxB=g["idxB"][c],
                         dstf=g["dstf"][c], esx=esx, edx=edx,
                         esself=ess, edself=eds, iota=iota, iotac=iotac, br=b2r))
    r4 = _run("a2", progs["a2"], ins4)
    hoT = [r4[c]["ho"] for c in range(NCORES)]
    h2out = _assemble_full(hoT, F_IN)

    # ---- L5: link predictor
    i0 = np.zeros((NCORES, 16, PT * 8), np.int16)
    i1 = np.zeros((NCORES, 16, PT * 8), np.int16)
    perm = np.zeros((NCORES, pc), np.int64)       # slot -> original pair row
    goff = np.concatenate([[0], np.cumsum(gt_sizes)]) * 128
    for c in range(NCORES):
        m0, m1, gk = lk_group[c]
        pos = 0
        slots = np.zeros(pc, np.int64)
        for q in range(4):
            sel = np.nonzero(gk == q)[0]
            slots[sel] = goff[q] + np.arange(len(sel))
        perm[c] = slots
        s0 = np.where(m0 >= RH, m0 - RH, m0).astype(np.int16)
        s1 = np.where(m1 >= RH, m1 - RH, m1).astype(np.int16)
        i0[c, slots % 16, slots // 16] = s0
        i1[c, slots % 16, slots // 16] = s1
    i0 = np.broadcast_to(i0[:, None], (NCORES, 8, 16, PT * 8)).reshape(NCORES, 128, -1)
    i1 = np.broadcast_to(i1[:, None], (NCORES, 8, 16, PT * 8)).reshape(NCORES, 128, -1)
    wl0 = _rep(Wl[:F_IN, 0])
    wl1 = _rep(Wl[F_IN:, 0])
    blr = np.full((128, 1), float(bl[0]), np.float32)
    r5 = _run("lk", progs["lk"], [
        dict(tableA=h2out[:RH], tableB=h2out[RH:],
             i0=np.ascontiguousarray(i0[c]), i1=np.ascontiguousarray(i1[c]),
             wl0=wl0, wl1=wl1, blr=blr)
        for c in range(NCORES)
    ])
    out = np.zeros((P, 1), np.float32)
    for c in range(NCORES):
        zc = r5[c]["z"]
        out[c * pc:(c + 1) * pc, 0] = zc[perm[c] % 128, perm[c] // 128]
    tot = sum(v for v in LAST_EXEC_NS.values() if v)
    print(f"kernel launches ns: {LAST_EXEC_NS} total {tot}")
    return out
